# revision 23
# baseline (speedup 1.0000x reference)
"""Trainium2 Bass kernel for a 3-layer GRU (B=512, T=512, D=22, H=64) + MLP head.

Strategy (data-parallel over batch, 64 rows/core on 8 cores):
  - Feature-major layout on-chip: h kept as [H, B_loc] so the recurrence
    needs no transposes. bf16 storage for all matmul inputs and gate
    tiles; PSUM accumulation stays fp32.
  - Biases applied without ones-row augmentation: the r|z' sigmoid bias
    rides the scalar-engine activation's per-partition bias operand; the
    n-gate biases (b_hn, b_in) ride scalar_tensor_tensor fusions on DVE.
  - 1-z computed directly as sigmoid of negated z-gate weights, so
    h' = h + (1-z) * (n - h) needs only 3 vector ops.
  - Per 4-step PSUM chunk: x-side matmuls fill the banks in bulk, then
    per-step h-side matmuls accumulate on top; sigma/tanh read PSUM.
  - The 3 layers run as a software pipeline interleaved at STEP
    granularity (engines see the three independent per-layer dependency
    chains back-to-back, which is what lets in-order engines overlap
    them).
  - MLP head (BN folded into the weights on host) runs on the final h.
"""
import numpy as np
from contextlib import ExitStack

B, T, D_IN, H = 512, 512, 22, 64
NCORES = 8
BL = B // NCORES          # 64 batch rows per core
CH = 4                    # time steps per PSUM chunk
EPS = 1e-5

_PROGRAM_CACHE = {}


def _np32(a):
    return np.ascontiguousarray(np.asarray(a), dtype=np.float32)


def _bf16(a):
    import ml_dtypes
    return np.ascontiguousarray(np.asarray(a).astype(ml_dtypes.bfloat16))


def _prep_weights(inp):
    """Transposed, gate-reordered bf16 weights + fp32 per-partition biases."""
    w = {}
    for l in range(3):
        w_ih = _np32(inp[f"w_ih{l}"])            # [192, din]
        w_hh = _np32(inp[f"w_hh{l}"])            # [192, 64]
        b_ih = _np32(inp[f"b_ih{l}"])            # [192]
        b_hh = _np32(inp[f"b_hh{l}"])            # [192]
        r, z, n = slice(0, H), slice(H, 2 * H), slice(2 * H, 3 * H)

        # x-side z'|r matmul weights: lhsT [din, 128]; z-gate negated so
        # sigmoid gives 1-z. Gate order is [z' | r]: z' must sit at base
        # partition 0 (it feeds an SBUF*SBUF DVE op, which requires equal
        # base partitions; r only meets PSUM operands, which are exempt).
        w[f"wxrz{l}"] = _bf16(np.concatenate([-w_ih[z].T, w_ih[r].T], axis=1))
        w[f"wxn{l}"] = _bf16(w_ih[n].T)
        w[f"whrz{l}"] = _bf16(np.concatenate([-w_hh[z].T, w_hh[r].T], axis=1))
        w[f"whn{l}"] = _bf16(w_hh[n].T)
        # sigmoid bias [128,1]: z' rows negated to match the negated weights
        brz = np.concatenate([-(b_ih[z] + b_hh[z]), b_ih[r] + b_hh[r]])
        w[f"brz{l}"] = _np32(brz).reshape(2 * H, 1)
        # b_hn parked at partitions 64-127 so the m1 STT's two SBUF inputs
        # (this scalar and sig[64:128]) share a base partition.
        bhn = np.zeros((2 * H, 1), np.float32)
        bhn[H:2 * H, 0] = b_hh[n]
        w[f"bhn{l}"] = bhn
        w[f"bin{l}"] = _np32(b_ih[n]).reshape(H, 1)

    def fold_bn(wf, bf, g, b_, m, v):
        s = g / np.sqrt(v + EPS)
        return wf * s[:, None], (bf - m) * s + b_

    w1, b1 = fold_bn(_np32(inp["fc1_w"]), _np32(inp["fc1_b"]), _np32(inp["bn1_g"]),
                     _np32(inp["bn1_b"]), _np32(inp["bn1_m"]), _np32(inp["bn1_v"]))
    w2, b2 = fold_bn(_np32(inp["fc2_w"]), _np32(inp["fc2_b"]), _np32(inp["bn2_g"]),
                     _np32(inp["bn2_b"]), _np32(inp["bn2_m"]), _np32(inp["bn2_v"]))
    w3, b3 = _np32(inp["fc3_w"]), _np32(inp["fc3_b"])

    w["fc1"] = _bf16(w1.T)                       # [64, 54]
    w["fb1"] = _np32(b1).reshape(54, 1)
    w["fc2"] = _bf16(w2.T)                       # [54, 44]
    w["fb2"] = _np32(b2).reshape(44, 1)
    w["fc3"] = _bf16(w3.T)                       # [44, 4]
    w["fb3"] = _np32(b3).reshape(4, 1)
    return w


def _prep_x_core(x_core):
    """x_core [BL, 22, T] -> xt [22, T*BL] feature-major bf16."""
    t = x_core.shape[2]
    return _bf16(_np32(x_core).transpose(1, 2, 0).reshape(D_IN, t * BL))


def _build(t_steps):
    import concourse.bacc as bacc
    import concourse.tile as tile
    from concourse import mybir

    f32 = mybir.dt.float32
    bf16 = mybir.dt.bfloat16
    AF = mybir.ActivationFunctionType
    ALU = mybir.AluOpType
    ts = __import__("concourse.bass", fromlist=["ts"]).ts

    nch = t_steps // CH
    nc = bacc.Bacc("TRN2", target_bir_lowering=False, debug=False)

    xt = nc.dram_tensor("xt", [D_IN, t_steps * BL], bf16, kind="ExternalInput").ap()
    wd = {}
    for l in range(3):
        din = D_IN if l == 0 else H
        wd[f"wxrz{l}"] = nc.dram_tensor(f"wxrz{l}", [din, 2 * H], bf16, kind="ExternalInput").ap()
        wd[f"wxn{l}"] = nc.dram_tensor(f"wxn{l}", [din, H], bf16, kind="ExternalInput").ap()
        wd[f"whrz{l}"] = nc.dram_tensor(f"whrz{l}", [H, 2 * H], bf16, kind="ExternalInput").ap()
        wd[f"whn{l}"] = nc.dram_tensor(f"whn{l}", [H, H], bf16, kind="ExternalInput").ap()
        wd[f"brz{l}"] = nc.dram_tensor(f"brz{l}", [2 * H, 1], f32, kind="ExternalInput").ap()
        wd[f"bhn{l}"] = nc.dram_tensor(f"bhn{l}", [2 * H, 1], f32, kind="ExternalInput").ap()
        wd[f"bin{l}"] = nc.dram_tensor(f"bin{l}", [H, 1], f32, kind="ExternalInput").ap()
    wd["fc1"] = nc.dram_tensor("fc1", [H, 54], bf16, kind="ExternalInput").ap()
    wd["fb1"] = nc.dram_tensor("fb1", [54, 1], f32, kind="ExternalInput").ap()
    wd["fc2"] = nc.dram_tensor("fc2", [54, 44], bf16, kind="ExternalInput").ap()
    wd["fb2"] = nc.dram_tensor("fb2", [44, 1], f32, kind="ExternalInput").ap()
    wd["fc3"] = nc.dram_tensor("fc3", [44, 4], bf16, kind="ExternalInput").ap()
    wd["fb3"] = nc.dram_tensor("fb3", [4, 1], f32, kind="ExternalInput").ap()
    y = nc.dram_tensor("y", [4, BL], f32, kind="ExternalOutput").ap()

    with tile.TileContext(nc) as tc, ExitStack() as ctx:
        const = ctx.enter_context(tc.tile_pool(name="const", bufs=1))
        xpool = ctx.enter_context(tc.tile_pool(name="xpool", bufs=3))
        hpools = [ctx.enter_context(tc.tile_pool(name=f"hseq{l}", bufs=3))
                  for l in range(3)]
        przp = [ctx.enter_context(tc.tile_pool(name=f"prz{l}", bufs=1, space="PSUM"))
                for l in range(3)]
        pnp = [ctx.enter_context(tc.tile_pool(name=f"pn{l}", bufs=1, space="PSUM"))
               for l in range(3)]
        mlpp = ctx.enter_context(tc.tile_pool(name="mlpp", bufs=2, space="PSUM"))
        sigp = ctx.enter_context(tc.tile_pool(name="sigp", bufs=6))
        m1p = ctx.enter_context(tc.tile_pool(name="m1p", bufs=6))
        nap = ctx.enter_context(tc.tile_pool(name="nap", bufs=6))
        ntp = ctx.enter_context(tc.tile_pool(name="ntp", bufs=6))
        aap = ctx.enter_context(tc.tile_pool(name="aap", bufs=6))
        bbp = ctx.enter_context(tc.tile_pool(name="bbp", bufs=6))
        mlps = ctx.enter_context(tc.tile_pool(name="mlps", bufs=1))

        # Load all weights.
        ws = {}
        for name, ap in wd.items():
            wt = const.tile(list(ap.shape), ap.dtype, tag=name)
            nc.sync.dma_start(out=wt, in_=ap)
            ws[name] = wt

        # Zero initial-state tile.
        zt = const.tile([H, BL], bf16, tag="zt")
        nc.vector.memset(zt, 0.0)

        hseq = [[None] * nch for _ in range(3)]
        state = {}

        def emit_xside(l, c):
            """DMA (layer 0) + x-side matmuls filling the PSUM banks."""
            kin = D_IN if l == 0 else H
            if l == 0:
                xc = xpool.tile([D_IN, CH * BL], bf16)
                nc.sync.dma_start(out=xc, in_=xt[:, c * CH * BL:(c + 1) * CH * BL])
                rhs_x = xc
            else:
                rhs_x = hseq[l - 1][c]
            hc = hpools[l].tile([H, CH * BL], bf16)
            hseq[l][c] = hc
            prz = przp[l].tile([2 * H, CH * BL], f32)
            pn = pnp[l].tile([H, 2 * CH * BL], f32)
            # x-side: i_r | i_z' into prz; i_n into pn[:, CH*BL:].
            nc.tensor.matmul(prz[:, :], ws[f"wxrz{l}"], rhs_x[0:kin, :],
                             start=True, stop=False, skip_group_check=True)
            nc.tensor.matmul(pn[:, CH * BL:2 * CH * BL], ws[f"wxn{l}"],
                             rhs_x[0:kin, :], start=True, stop=True,
                             skip_group_check=True)
            state[l] = (prz, pn, hc)

        def hprev(l, c, j):
            if c == 0 and j == 0:
                return zt
            if j == 0:
                return hseq[l][c - 1][:, ts(CH - 1, BL)]
            return state[l][2][:, ts(j - 1, BL)]

        def emit_mm(l, c, j):
            prz, pn, hc = state[l]
            hp = hprev(l, c, j)
            nc.tensor.matmul(prz[:, ts(j, BL)], ws[f"whrz{l}"], hp,
                             start=False, stop=True, skip_group_check=True)
            nc.tensor.matmul(pn[:, ts(j, BL)], ws[f"whn{l}"], hp,
                             start=True, stop=True, skip_group_check=True)

        def emit_sig(l, c, j):
            prz = state[l][0]
            sig = sigp.tile([2 * H, BL], bf16)
            nc.scalar.activation(sig, prz[:, ts(j, BL)], AF.Sigmoid,
                                 bias=ws[f"brz{l}"][:, 0:1])
            return sig

        def emit_m1na(l, c, j, sig):
            pn = state[l][1]
            # m1 = (hn + b_hn) * r
            m1 = m1p.tile([H, BL], bf16)
            nc.vector.scalar_tensor_tensor(m1, pn[:, ts(j, BL)],
                                           ws[f"bhn{l}"][H:2 * H, 0:1],
                                           sig[H:2 * H, :],
                                           ALU.add, ALU.mult)
            # na = (i_n + b_in) + m1, written back into the hn PSUM slot
            # (already consumed by m1) so tanh reads the scalar engine's
            # cheaper PSUM port
            nc.vector.scalar_tensor_tensor(pn[:, ts(j, BL)],
                                           pn[:, ts(CH + j, BL)],
                                           ws[f"bin{l}"][:, 0:1], m1,
                                           ALU.add, ALU.add)
            return None

        def emit_tanh(l, c, j, na):
            pn = state[l][1]
            nt = ntp.tile([H, BL], bf16)
            nc.scalar.activation(nt, pn[:, ts(j, BL)], AF.Tanh)
            return nt

        def emit_update(l, c, j, sig, nt):
            hc = state[l][2]
            hp = hprev(l, c, j)
            aa = aap.tile([H, BL], bf16)
            nc.vector.tensor_sub(aa, nt, hp)
            bb = bbp.tile([H, BL], bf16)
            nc.vector.tensor_mul(bb, sig[0:H, :], aa)
            nc.vector.tensor_add(hc[:, ts(j, BL)], hp, bb)

        for g in range(nch + 2):
            active = [(l, g - l) for l in range(3) if 0 <= g - l < nch]
            for l, c in active:
                emit_xside(l, c)
            for j in range(CH):
                sigs, nas, nts = {}, {}, {}
                for l, c in active:
                    emit_mm(l, c, j)
                for l, c in active:
                    sigs[l] = emit_sig(l, c, j)
                for l, c in active:
                    nas[l] = emit_m1na(l, c, j, sigs[l])
                for l, c in active:
                    nts[l] = emit_tanh(l, c, j, nas[l])
                for l, c in active:
                    emit_update(l, c, j, sigs[l], nts[l])

        # MLP head on the last hidden state of layer 2.
        hlast = hseq[2][nch - 1][:, ts(CH - 1, BL)]
        pm1 = mlpp.tile([54, BL], f32, tag="mlp")
        nc.tensor.matmul(pm1, ws["fc1"], hlast, start=True, stop=True)
        y1 = mlps.tile([54, BL], bf16, tag="y1")
        nc.scalar.activation(y1, pm1, AF.Relu, bias=ws["fb1"][:, 0:1])
        pm2 = mlpp.tile([44, BL], f32, tag="mlp")
        nc.tensor.matmul(pm2, ws["fc2"], y1, start=True, stop=True)
        y2 = mlps.tile([44, BL], bf16, tag="y2")
        nc.scalar.activation(y2, pm2, AF.Relu, bias=ws["fb2"][:, 0:1])
        pm3 = mlpp.tile([4, BL], f32, tag="mlp")
        nc.tensor.matmul(pm3, ws["fc3"], y2, start=True, stop=True)
        yo = mlps.tile([4, BL], f32, tag="yo")
        nc.vector.tensor_scalar_add(yo, pm3, ws["fb3"][:, 0:1])
        nc.sync.dma_start(out=y, in_=yo)

    nc.compile()
    return nc


def get_program(t_steps=T):
    if t_steps not in _PROGRAM_CACHE:
        _PROGRAM_CACHE[t_steps] = _build(t_steps)
    return _PROGRAM_CACHE[t_steps]


def make_in_maps(inputs, t_steps=T):
    x = _np32(inputs["x"])
    w = _prep_weights(inputs)
    in_maps = []
    for c in range(NCORES):
        m = dict(w)
        m["xt"] = _prep_x_core(x[c * BL:(c + 1) * BL, :, :t_steps])
        in_maps.append(m)
    return in_maps


def kernel(**inputs) -> np.ndarray:
    from concourse.bass_utils import run_bass_kernel_spmd

    nc = get_program(T)
    in_maps = make_in_maps(inputs, T)
    res = run_bass_kernel_spmd(nc, in_maps, list(range(NCORES)))
    out = np.empty((B, 4), np.float32)
    for c in range(NCORES):
        out[c * BL:(c + 1) * BL] = res.results[c]["y"].T
    return out
